# revision 16
# baseline (speedup 1.0000x reference)
"""Trainium2 Bass kernel for GPT-NeoX-style attention block (nn_Attention_88141318848873).

Full inputs -> head-parallel tensor-parallel across 8 NeuronCores -> full output.

Fused single-stream design (v2). Per core c (local heads 4c..4c+3):
  - One unified instruction stream: for each 512-token q-block, the QKV
    projection of the NEXT block and the out-projection of the PREVIOUS
    block are interleaved (via round-robin generators) into the attention
    ki-steps of the CURRENT block. This keeps the PE continuously busy
    (p-state stays at max clock), keeps ScalarE's exp stream fed from
    ~5% into the kernel, and pads the short PV matmuls with long
    projection matmuls so PV weight loads hide under real work.
  - PSUM plan (8 banks, all pools open for the whole kernel):
    qk-proj 1 (bufs=1) + v-proj 1 (manual ping-pong halves) + scores 2
    (per-head 1-bank tiles, 2 bufs) + PV acc 2 + out-proj 1 + transposes 1
    (single [128,4,128] slot shared by qk-transposes and attnN-transposes).
  - Host pre-tiles every DRAM operand so each DMA is per-partition
    contiguous (the baseline's strided xT reads cost a 21.5us startup
    stall and ~2x DMA inefficiency).
  - Attention core unchanged from v1: scores transposed S^T[k,q] with
    2 heads packed via tile_position; no-max-sub softmax; exp on ScalarE,
    causal 0/1 mask applied post-exp on DVE; unnormalized P with
    ones-augmented V giving denominators; per-q-tile PV accumulation
    opened by one whole-tile zeroing matmul per head.
Host: shards/pre-transposes/casts inputs (scale 1/sqrt(hd) folded into Wq,
rope cos/sin tables partition-linear), sums the 8 bf16 partial outputs in f32.
"""
import sys

sys.path.insert(0, "/opt/trn_rl_repo")

import numpy as np
import ml_dtypes

import concourse.bass as bass
import concourse.mybir as mybir
import concourse.tile as tile
from concourse.bacc import Bacc
from concourse.bass_utils import run_bass_kernel_spmd
from concourse.masks import make_identity

B, S_FULL, H = 2, 2048, 2048
NH, HD, ROT = 32, 64, 16
THETA = 10000.0
NCORES = 8
HPC = NH // NCORES            # heads per core = 4
LDIM = HPC * HD               # local attn dims = 256
NEG = -1e30

bf16 = mybir.dt.bfloat16
f32 = mybir.dt.float32
nbf16 = ml_dtypes.bfloat16
Exp = mybir.ActivationFunctionType.Exp


# --------------------------------------------------------------------------
# Bass program (identical on every core; per-core tensors differ)
# --------------------------------------------------------------------------

def build_nc(S=S_FULL):
    assert S % 512 == 0
    T = B * S
    TT = T // 128                 # token tiles total
    TPB = S // 128                # token tiles per batch
    NQB = S // 512                # 512-wide q blocks per batch
    HC = H // 128                 # h (contraction) chunks

    nc = Bacc()
    # all DRAM tensors pre-tiled on host: partition dim second/first so each
    # DMA is per-partition contiguous
    xt_d = nc.dram_tensor("xtl", [TT, 128, HC, 128], bf16, kind="ExternalInput")
    wqkv_d = nc.dram_tensor("wqkvl", [128, HC, 768], bf16, kind="ExternalInput")
    wo_d = nc.dram_tensor("wol", [128, 2, H], bf16, kind="ExternalInput")
    cs_d = nc.dram_tensor("csd", [128, TT, 2 * ROT], bf16, kind="ExternalInput")
    mask_d = nc.dram_tensor("maskd", [128, 128], bf16, kind="ExternalInput")
    out_d = nc.dram_tensor("out", [T, H], bf16, kind="ExternalOutput")

    blocks = [(b, qb) for b in range(B) for qb in range(NQB)]

    with tile.TileContext(nc) as tc:
        with tc.tile_pool(name="const", bufs=1) as cpool:
            wqkv_sb = cpool.tile([128, HC, 768], bf16)
            # per-hc chunks: the first matmul only needs chunk 0
            wo_sb = cpool.tile([128, 2, H], bf16)
            cs_sb = cpool.tile([128, TT, 2 * ROT], bf16)
            mask_sb = cpool.tile([128, 128], bf16)   # 0/1 keep-mask, post-exp
            nc.sync.dma_start(out=mask_sb, in_=mask_d[:, :])
            ident = cpool.tile([128, 128], bf16)
            make_identity(nc, ident)
            zeros_sb = cpool.tile([128, 512], bf16)
            nc.vector.memset(zeros_sb, 0.0)

            qkT_sb = cpool.tile([128, 4, T], bf16)     # dims x tok (4 dtiles)
            V_sb = cpool.tile([128, TT, HPC, 66], bf16)  # tok x head x (64+one)
            nc.vector.memset(V_sb[:, :, :, 64:65], 1.0)
            attnN_sb = cpool.tile([128, 2, T], bf16)   # normalized attn

            sb_pools = [
                tc.tile_pool(name="xt", bufs=9),
                tc.tile_pool(name="qknat", bufs=3),
                tc.tile_pool(name="ropetmp", bufs=4),
                tc.tile_pool(name="ppool", bufs=6),
                tc.tile_pool(name="anpool", bufs=4),
                tc.tile_pool(name="recpool", bufs=4),
                tc.tile_pool(name="obpool", bufs=6),
            ]
            xpool, qpool, rpool, ppool, anpool, recpool, obpool = \
                [p.__enter__() for p in sb_pools]

            # PSUM: exactly 8 banks, all pools open the whole kernel.
            ps_pools = [
                tc.tile_pool(name="qkps", bufs=1, space="PSUM"),
                tc.tile_pool(name="vps", bufs=1, space="PSUM"),
                tc.tile_pool(name="spool", bufs=1, space="PSUM"),
                tc.tile_pool(name="apool", bufs=2, space="PSUM"),
                tc.tile_pool(name="opool", bufs=1, space="PSUM"),
                tc.tile_pool(name="tpool", bufs=1, space="PSUM"),
            ]
            qkpool, vpool, spool, apool, opool, tpool = \
                [p.__enter__() for p in ps_pools]

            # v projection double-buffers inside ONE bank (manual halves)
            v_ps = vpool.tile([128, 2, 256], f32)

            xt_tiles = {}

            def issue_block_dmas(bi, hp=False):
                b, qb = blocks[bi]
                for t4 in range(4):
                    ti = b * TPB + qb * 4 + t4
                    xt = xpool.tile([128, HC, 128], bf16, tag="xt")
                    if hp:
                        # split per-hc-chunk so the first matmuls start after
                        # ~128KB instead of the full 512KB tile
                        with tc.high_priority():
                            for hq in range(4):
                                nc.sync.dma_start(
                                    out=xt[:, hq * 4:(hq + 1) * 4, :],
                                    in_=xt_d[ti, :, hq * 4:(hq + 1) * 4, :])
                    else:
                        nc.sync.dma_start(out=xt, in_=xt_d[ti])
                    xt_tiles[ti] = xt

            def issue_startup_dmas():
                """First block: wqkv/x chunks interleaved in consumption
                order so matmul (ti0, hc0) starts after ~300KB, not ~5MB."""
                ti0 = blocks[0][0] * TPB + blocks[0][1] * 4
                xts = [xpool.tile([128, HC, 128], bf16, tag="xt", name=f"xt{t}")
                       for t in range(4)]
                with tc.high_priority():
                    for hq in range(4):
                        for hcd in range(hq * 4, hq * 4 + 4):
                            nc.sync.dma_start(
                                out=wqkv_sb[:, hcd, :], in_=wqkv_d[:, hcd, :])
                        nc.sync.dma_start(
                            out=xts[0][:, hq * 4:(hq + 1) * 4, :],
                            in_=xt_d[ti0, :, hq * 4:(hq + 1) * 4, :])
                    nc.sync.dma_start(out=cs_sb, in_=cs_d[:, :, :])
                    for t in range(1, 4):
                        for hq in range(4):
                            nc.sync.dma_start(
                                out=xts[t][:, hq * 4:(hq + 1) * 4, :],
                                in_=xt_d[ti0 + t, :, hq * 4:(hq + 1) * 4, :])
                for t in range(4):
                    xt_tiles[ti0 + t] = xts[t]

            def proj_stream(bi):
                """QKV projection + RoPE + transpose for one 512-tok block.
                Yields after each PE instruction (interleave granularity)."""
                b, qb = blocks[bi]
                for t4 in range(4):
                    ti = b * TPB + qb * 4 + t4
                    xt = xt_tiles.pop(ti)
                    qk_ps = qkpool.tile([128, 512], f32, tag="qk")
                    for hc in range(HC):
                        nc.tensor.matmul(
                            qk_ps, xt[:, hc, :], wqkv_sb[:, hc, 0:512],
                            start=(hc == 0), stop=(hc == HC - 1))
                        yield
                    vh = v_ps[:, ti % 2, :]
                    for hc in range(HC):
                        nc.tensor.matmul(
                            vh, xt[:, hc, :], wqkv_sb[:, hc, 512:768],
                            start=(hc == 0), stop=(hc == HC - 1))
                        yield
                    # V -> SBUF (bf16), ones col already set
                    nc.vector.tensor_copy(
                        V_sb[:, ti, :, 0:64],
                        vh.rearrange("p (h d) -> p h d", d=64))
                    # qk -> SBUF bf16
                    qk = qpool.tile([128, 512], bf16, tag="qk")
                    nc.vector.tensor_copy(qk, qk_ps)
                    # partial RoPE on dims 0..15 of the 8 (q/k, head) blocks
                    rot = qk.rearrange("p (b d) -> p b d", d=64)[:, :, 0:ROT]
                    rot_lo = qk.rearrange("p (b d) -> p b d", d=64)[:, :, 0:8]
                    rot_hi = qk.rearrange("p (b d) -> p b d", d=64)[:, :, 8:16]
                    cos_bc = cs_sb[:, ti, None, 0:ROT].broadcast_to([128, 8, ROT])
                    sin_lo = cs_sb[:, ti, None, ROT:ROT + 8].broadcast_to([128, 8, 8])
                    sin_hi = cs_sb[:, ti, None, ROT + 8:ROT + 16].broadcast_to([128, 8, 8])
                    tmp = rpool.tile([128, 8, ROT], bf16, tag="t0")
                    t2l = rpool.tile([128, 8, 8], bf16, tag="t1")
                    t2h = rpool.tile([128, 8, 8], bf16, tag="t2")
                    nc.vector.tensor_mul(tmp, rot, cos_bc)
                    nc.vector.tensor_mul(t2l, rot_hi, sin_lo)
                    nc.vector.tensor_mul(t2h, rot_lo, sin_hi)
                    nc.vector.tensor_sub(rot_lo, tmp[:, :, 0:8], t2l)
                    nc.vector.tensor_add(rot_hi, tmp[:, :, 8:16], t2h)
                    # transpose the 4 dim-tiles into qkT
                    # all copies on DVE: a scalar.copy here would head-of-line
                    # block the exp stream on ScalarE's FIFO queue
                    tp = tpool.tile([128, 4, 128], bf16, tag="tp")
                    for dt in range(4):
                        nc.tensor.transpose(
                            tp[:, dt, :], qk[:, dt * 128:(dt + 1) * 128], ident)
                        nc.vector.tensor_copy(
                            qkT_sb[:, dt, ti * 128:(ti + 1) * 128],
                            tp[:, dt, :])
                        yield

            def oproj_stream(bi, use_act=False):
                """Out-projection for one completed 512-tok block."""
                b, qb = blocks[bi]
                for t4 in range(4):
                    ti = b * TPB + qb * 4 + t4
                    tp = tpool.tile([128, 4, 128], bf16, tag="tp")
                    nc.tensor.transpose(
                        tp[:, 0, :],
                        attnN_sb[:, 0, ti * 128:(ti + 1) * 128], ident)
                    nc.tensor.transpose(
                        tp[:, 1, :],
                        attnN_sb[:, 1, ti * 128:(ti + 1) * 128], ident)
                    aT = anpool.tile([128, 2, 128], bf16, tag="an")
                    nc.vector.tensor_copy(aT, tp[:, 0:2, :])
                    yield
                    for oc in range(4):
                        ops = opool.tile([128, 512], f32, tag="o")
                        nc.tensor.matmul(
                            ops, aT[:, 0, :],
                            wo_sb[:, 0, oc * 512:(oc + 1) * 512],
                            start=True, stop=False)
                        yield
                        nc.tensor.matmul(
                            ops, aT[:, 1, :],
                            wo_sb[:, 1, oc * 512:(oc + 1) * 512],
                            start=False, stop=True)
                        yield
                        ob = obpool.tile([128, 512], bf16, tag="ob")
                        if use_act and oc % 2 == 1:
                            nc.scalar.copy(ob, ops)
                        else:
                            nc.vector.tensor_copy(ob, ops)
                        nc.sync.dma_start(
                            out=out_d[ti * 128:(ti + 1) * 128,
                                      oc * 512:(oc + 1) * 512],
                            in_=ob)

            fills = []
            pending = [0]          # fill units outstanding
            pace = [0.0, 1.0]      # [accumulator, units-per-slot rate]

            def add_fill(gen, units):
                fills.append(gen)
                pending[0] += units

            def pull(n):
                """Advance fill generators round-robin, n units total."""
                for _ in range(n):
                    while fills:
                        try:
                            next(fills[0])
                            pending[0] -= 1
                            fills.append(fills.pop(0))
                            break
                        except StopIteration:
                            fills.pop(0)
                    if not fills:
                        return

            def paced_pull(weight):
                """Offer `weight` interleave slots; pull at the paced rate so
                fill work lasts the whole block instead of front-loading."""
                pace[0] += weight * pace[1]
                k = int(pace[0])
                if k > 0:
                    pace[0] -= k
                    pull(k)

            def drain():
                while fills:
                    pull(1)

            def force_finish(g):
                """Emit every remaining unit of one generator now."""
                n = 0
                for _ in g:
                    n += 1
                pending[0] -= n

            proj_gens = {}
            VJ = [10, 26, 42, 58]   # sum of valid (ki, j) pairs per qb

            # ---------------- prologue: project block 0 standalone
            issue_startup_dmas()
            issue_block_dmas(1, hp=True)
            wo_loaded = [False]
            proj_gens[0] = proj_stream(0)
            for _ in proj_gens[0]:
                pass

            nc.sync.dma_start(out=wo_sb, in_=wo_d[:, :, :])

            # ---------------- unified stream
            for bi, (b, qb) in enumerate(blocks):
                if bi + 1 < len(blocks):
                    if bi + 2 < len(blocks):
                        issue_block_dmas(bi + 2)
                    proj_gens[bi + 1] = proj_stream(bi + 1)
                    add_fill(proj_gens[bi + 1], 144)
                # attention(bi) depends on projection(bi): its instructions
                # must already be emitted (program order defines deps)
                if bi in proj_gens:
                    force_finish(proj_gens.pop(bi))
                # pace fills across this block's interleave slots
                slots = 2 * ((4 * qb + 4) * 2 + 2 * VJ[qb] + 8)
                pace[1] = pending[0] / slots if slots else 1.0
                pace[0] = 0.0
                for pr in range(2):          # head pairs (2pr, 2pr+1)
                    hA, hB = 2 * pr, 2 * pr + 1
                    accA = apool.tile([128, 4, 65], f32, tag="acc")
                    accB = apool.tile([128, 4, 65], f32, tag="acc")
                    with tc.high_priority(offset=150):
                        nc.tensor.matmul(
                            accA, ident, zeros_sb[:, 0:260],
                            start=True, stop=False)
                        nc.tensor.matmul(
                            accB, ident, zeros_sb[:, 0:260],
                            start=True, stop=False)
                    for ki in range(4 * qb + 4):
                        off = max(0, ki * 128 - qb * 512)
                        kcol = b * S + ki * 128
                        qcol = b * S + qb * 512
                        # one 2-bank tile + one exp per step: both score
                        # matmuls wait on the same exp, so the tile_position
                        # pair always runs back-to-back (473ns vs 2x318 split)
                        sAB = spool.tile([128, 2, 512], f32, tag="s")
                        with tc.high_priority(offset=150):
                            nc.tensor.matmul(
                                sAB[:, 0, off:512],
                                qkT_sb[0:64, 2 + pr, kcol:kcol + 128],
                                qkT_sb[0:64, pr, qcol + off:qcol + 512],
                                start=True, stop=True,
                                tile_position=(0, 0))
                            nc.tensor.matmul(
                                sAB[:, 1, off:512],
                                qkT_sb[64:128, 2 + pr, kcol:kcol + 128],
                                qkT_sb[64:128, pr, qcol + off:qcol + 512],
                                start=True, stop=True,
                                tile_position=(64, 0))
                        pAB = ppool.tile([128, 2, 512], bf16, tag="p")
                        nc.scalar.activation(
                            out=pAB[:, :, off:512], in_=sAB[:, :, off:512],
                            func=Exp)
                        if ki * 128 >= qb * 512:  # in-block diagonal
                            mask2 = mask_sb[:, None, :].broadcast_to(
                                [128, 2, 128])
                            nc.vector.tensor_mul(
                                pAB[:, :, off:off + 128],
                                pAB[:, :, off:off + 128], mask2)
                        paced_pull(2)
                        for j in range(4):
                            qg = qb * 4 + j
                            if qg < ki:
                                continue
                            last = (j == 3 and ki == 4 * qb + 3)
                            nc.tensor.matmul(
                                accA[:, j, 0:65],
                                pAB[:, 0, j * 128:(j + 1) * 128],
                                V_sb[:, b * TPB + ki, hA, 0:65],
                                start=False, stop=last)
                            paced_pull(1)
                            nc.tensor.matmul(
                                accB[:, j, 0:65],
                                pAB[:, 1, j * 128:(j + 1) * 128],
                                V_sb[:, b * TPB + ki, hB, 0:65],
                                start=False, stop=last)
                            paced_pull(1)
                    # normalize into attnN (q x dims layout)
                    last_block = (bi == len(blocks) - 1)
                    inline_op = None
                    if False and last_block and pr == 1:
                        inline_op = oproj_stream(bi, use_act=True)
                    for j in range(4):
                        recA = recpool.tile([128, 1], f32, tag="r")
                        recB = recpool.tile([128, 1], f32, tag="r")
                        nc.vector.reciprocal(recA, accA[:, j, 64:65])
                        nc.vector.reciprocal(recB, accB[:, j, 64:65])
                        col = b * S + (qb * 4 + j) * 128
                        nc.vector.tensor_scalar_mul(
                            attnN_sb[:, pr, col:col + 64],
                            accA[:, j, 0:64], recA)
                        nc.vector.tensor_scalar_mul(
                            attnN_sb[:, pr, col + 64:col + 128],
                            accB[:, j, 0:64], recB)
                        if inline_op is not None:
                            # last block: emit this q-tile's out-projection
                            # right after its attnN columns finalize
                            for _ in range(9):
                                try:
                                    next(inline_op)
                                except StopIteration:
                                    break
                        else:
                            paced_pull(2)
                    if inline_op is not None:
                        for _ in inline_op:
                            pass
                # out-projection of this block interleaves into the next
                # block's attention; the last block was emitted inline above
                if not last_block:
                    add_fill(oproj_stream(bi), 36)
                else:
                    add_fill(oproj_stream(bi, use_act=True), 36)
                    drain()

            for p in reversed(ps_pools):
                p.__exit__(None, None, None)
            for p in reversed(sb_pools):
                p.__exit__(None, None, None)
    nc.finalize()
    return nc


# --------------------------------------------------------------------------
# Host-side prep
# --------------------------------------------------------------------------

def _host_prep(hidden_states, qkv_w, o_w, position_ids, S=S_FULL):
    """Returns (shared dict, per-core list of dicts) of numpy arrays."""
    T = B * S
    TT = T // 128
    HC = H // 128
    x = np.asarray(hidden_states, dtype=np.float32).reshape(T, H)
    # pre-tiled xT: [TT, 128(p=h%128), HC, 128(tok)] contiguous per DMA tile
    xtl = np.ascontiguousarray(
        x.reshape(TT, 128, HC, 128).transpose(0, 3, 2, 1)).astype(nbf16)

    pos = np.asarray(position_ids).reshape(T).astype(np.float64)
    inv = THETA ** (-np.arange(0, ROT, 2, dtype=np.float64) / ROT)  # [8]
    f = pos[:, None] * inv[None, :]                                 # [T, 8]
    emb = np.concatenate([f, f], axis=1)                            # [T, 16]
    # packed per-partition-linear layout [128, TT, 32]: cos | sin
    cs = np.empty((128, TT, 2 * ROT), np.float32)
    cs[:, :, 0:ROT] = np.cos(emb).reshape(TT, 128, ROT).transpose(1, 0, 2)
    cs[:, :, ROT:2 * ROT] = np.sin(emb).reshape(TT, 128, ROT).transpose(1, 0, 2)
    csd = np.ascontiguousarray(cs).astype(nbf16)

    # mask[p, j]: 1 when q offset j >= k offset p else 0 (applied post-exp)
    p_idx = np.arange(128)[:, None]
    j_idx = np.arange(128)[None, :]
    maskd = np.ascontiguousarray(
        np.where(j_idx >= p_idx, 1.0, 0.0)).astype(nbf16)

    shared = {"xtl": xtl, "csd": csd, "maskd": maskd}

    qkv = np.asarray(qkv_w, dtype=np.float32)
    ow = np.asarray(o_w, dtype=np.float32)
    scale = 1.0 / np.sqrt(HD)
    per_core = []
    for c in range(NCORES):
        cols = np.empty((768, H), np.float32)
        for t in range(4):                    # qk dim-tiles
            qk_sel = 0 if t < 2 else 1        # 0 = q, 1 = k
            for u in range(2):
                hl = 2 * (t % 2) + u
                hg = HPC * c + hl
                w = qkv[qk_sel * H + hg * HD: qk_sel * H + (hg + 1) * HD]
                if qk_sel == 0:
                    w = w * scale
                cols[t * 128 + u * 64: t * 128 + u * 64 + 64] = w
        for hl in range(HPC):                 # v dims
            hg = HPC * c + hl
            cols[512 + hl * 64: 512 + (hl + 1) * 64] = \
                qkv[2 * H + hg * HD: 2 * H + (hg + 1) * HD]
        wqkvT = cols.T                        # [H, 768]
        wqkvl = np.ascontiguousarray(
            wqkvT.reshape(HC, 128, 768).transpose(1, 0, 2)).astype(nbf16)
        woT = ow[:, LDIM * c: LDIM * (c + 1)].T   # [256, H]
        wol = np.ascontiguousarray(
            woT.reshape(2, 128, H).transpose(1, 0, 2)).astype(nbf16)
        per_core.append({"wqkvl": wqkvl, "wol": wol})
    return shared, per_core


_NC_CACHE = {}


def _get_nc(S=S_FULL):
    if S not in _NC_CACHE:
        _NC_CACHE[S] = build_nc(S)
    return _NC_CACHE[S]


def _run(hidden_states, qkv_w, o_w, position_ids, S=S_FULL, trace=False,
         trace_kwargs=None):
    shared, per_core = _host_prep(hidden_states, qkv_w, o_w, position_ids, S)
    in_maps = [{**shared, **per_core[c]} for c in range(NCORES)]
    nc = _get_nc(S)
    br = run_bass_kernel_spmd(
        nc, in_maps, list(range(NCORES)), trace=trace,
        **(trace_kwargs or {}))
    T = B * S
    out = np.zeros((T, H), np.float32)
    for r in br.results:
        out += r["out"].astype(np.float32)
    return out.reshape(B, S, H), br


def kernel(hidden_states, qkv_w, o_w, position_ids):
    out, _ = _run(hidden_states, qkv_w, o_w, position_ids)
    return out


# revision 17
# speedup vs baseline: 1.2012x; 1.2012x over previous
"""Trainium2 Bass kernel for GPT-NeoX-style attention block (nn_Attention_88141318848873).

Full inputs -> head-parallel tensor-parallel across 8 NeuronCores -> full output.

Fused single-stream design (v2). Per core c (local heads 4c..4c+3):
  - One unified instruction stream: for each 512-token q-block, the QKV
    projection of the NEXT block and the out-projection of the PREVIOUS
    block are interleaved (via round-robin generators) into the attention
    ki-steps of the CURRENT block. This keeps the PE continuously busy
    (p-state stays at max clock), keeps ScalarE's exp stream fed from
    ~5% into the kernel, and pads the short PV matmuls with long
    projection matmuls so PV weight loads hide under real work.
  - PSUM plan (8 banks, all pools open for the whole kernel):
    qk-proj 1 (bufs=1) + v-proj 1 (manual ping-pong halves) + scores 2
    (per-head 1-bank tiles, 2 bufs) + PV acc 2 + out-proj 1 + transposes 1
    (single [128,4,128] slot shared by qk-transposes and attnN-transposes).
  - Host pre-tiles every DRAM operand so each DMA is per-partition
    contiguous (the baseline's strided xT reads cost a 21.5us startup
    stall and ~2x DMA inefficiency).
  - Attention core unchanged from v1: scores transposed S^T[k,q] with
    2 heads packed via tile_position; no-max-sub softmax; exp on ScalarE,
    causal 0/1 mask applied post-exp on DVE; unnormalized P with
    ones-augmented V giving denominators; per-q-tile PV accumulation
    opened by one whole-tile zeroing matmul per head.
Host: shards/pre-transposes/casts inputs (scale 1/sqrt(hd) folded into Wq,
rope cos/sin tables partition-linear), sums the 8 bf16 partial outputs in f32.
"""
import sys

sys.path.insert(0, "/opt/trn_rl_repo")

import numpy as np
import ml_dtypes

import concourse.bass as bass
import concourse.mybir as mybir
import concourse.tile as tile
from concourse.bacc import Bacc
from concourse.bass_utils import run_bass_kernel_spmd
from concourse.masks import make_identity

B, S_FULL, H = 2, 2048, 2048
NH, HD, ROT = 32, 64, 16
THETA = 10000.0
NCORES = 8
HPC = NH // NCORES            # heads per core = 4
LDIM = HPC * HD               # local attn dims = 256
NEG = -1e30

bf16 = mybir.dt.bfloat16
f32 = mybir.dt.float32
nbf16 = ml_dtypes.bfloat16
Exp = mybir.ActivationFunctionType.Exp


# --------------------------------------------------------------------------
# Bass program (identical on every core; per-core tensors differ)
# --------------------------------------------------------------------------

def build_nc(S=S_FULL):
    assert S % 512 == 0
    T = B * S
    TT = T // 128                 # token tiles total
    TPB = S // 128                # token tiles per batch
    NQB = S // 512                # 512-wide q blocks per batch
    HC = H // 128                 # h (contraction) chunks

    nc = Bacc()
    # all DRAM tensors pre-tiled on host: partition dim second/first so each
    # DMA is per-partition contiguous
    xt_d = nc.dram_tensor("xtl", [TT, 128, HC, 128], bf16, kind="ExternalInput")
    wqkv_d = nc.dram_tensor("wqkvl", [128, HC, 768], bf16, kind="ExternalInput")
    wo_d = nc.dram_tensor("wol", [128, 2, H], bf16, kind="ExternalInput")
    cs_d = nc.dram_tensor("csd", [128, TT, 2 * ROT], bf16, kind="ExternalInput")
    mask_d = nc.dram_tensor("maskd", [128, 128], bf16, kind="ExternalInput")
    out_d = nc.dram_tensor("out", [T, H], bf16, kind="ExternalOutput")

    blocks = [(b, qb) for b in range(B) for qb in range(NQB)]

    with tile.TileContext(nc) as tc:
        with tc.tile_pool(name="const", bufs=1) as cpool:
            wqkv_sb = cpool.tile([128, HC, 768], bf16)
            # per-hc chunks: the first matmul only needs chunk 0
            wo_sb = cpool.tile([128, 2, H], bf16)
            cs_sb = cpool.tile([128, TT, 2 * ROT], bf16)
            mask_sb = cpool.tile([128, 128], bf16)   # 0/1 keep-mask, post-exp
            nc.sync.dma_start(out=mask_sb, in_=mask_d[:, :])
            ident = cpool.tile([128, 128], bf16)
            make_identity(nc, ident)
            zeros_sb = cpool.tile([128, 512], bf16)
            nc.vector.memset(zeros_sb, 0.0)

            qkT_sb = cpool.tile([128, 4, T], bf16)     # dims x tok (4 dtiles)
            V_sb = cpool.tile([128, TT, HPC, 66], bf16)  # tok x head x (64+one)
            nc.vector.memset(V_sb[:, :, :, 64:65], 1.0)
            attnN_sb = cpool.tile([128, 2, T], bf16)   # normalized attn

            sb_pools = [
                tc.tile_pool(name="xt", bufs=9),
                tc.tile_pool(name="qknat", bufs=3),
                tc.tile_pool(name="ropetmp", bufs=4),
                tc.tile_pool(name="ppool", bufs=6),
                tc.tile_pool(name="anpool", bufs=4),
                tc.tile_pool(name="recpool", bufs=4),
                tc.tile_pool(name="obpool", bufs=6),
            ]
            xpool, qpool, rpool, ppool, anpool, recpool, obpool = \
                [p.__enter__() for p in sb_pools]

            # PSUM: exactly 8 banks, all pools open the whole kernel.
            ps_pools = [
                tc.tile_pool(name="qkps", bufs=1, space="PSUM"),
                tc.tile_pool(name="vps", bufs=1, space="PSUM"),
                tc.tile_pool(name="spool", bufs=1, space="PSUM"),
                tc.tile_pool(name="apool", bufs=2, space="PSUM"),
                tc.tile_pool(name="opool", bufs=1, space="PSUM"),
                tc.tile_pool(name="tpool", bufs=1, space="PSUM"),
            ]
            qkpool, vpool, spool, apool, opool, tpool = \
                [p.__enter__() for p in ps_pools]

            # v projection double-buffers inside ONE bank (manual halves)
            v_ps = vpool.tile([128, 2, 256], f32)

            xt_tiles = {}

            def issue_block_dmas(bi, hp=False):
                b, qb = blocks[bi]
                for t4 in range(4):
                    ti = b * TPB + qb * 4 + t4
                    xt = xpool.tile([128, HC, 128], bf16, tag="xt")
                    if hp:
                        # split per-hc-chunk so the first matmuls start after
                        # ~128KB instead of the full 512KB tile
                        with tc.high_priority():
                            for hq in range(4):
                                nc.sync.dma_start(
                                    out=xt[:, hq * 4:(hq + 1) * 4, :],
                                    in_=xt_d[ti, :, hq * 4:(hq + 1) * 4, :])
                    else:
                        nc.sync.dma_start(out=xt, in_=xt_d[ti])
                    xt_tiles[ti] = xt

            def issue_startup_dmas():
                """First block: wqkv/x chunks interleaved in consumption
                order so matmul (ti0, hc0) starts after ~300KB, not ~5MB."""
                ti0 = blocks[0][0] * TPB + blocks[0][1] * 4
                xts = [xpool.tile([128, HC, 128], bf16, tag="xt", name=f"xt{t}")
                       for t in range(4)]
                with tc.high_priority():
                    for hq in range(4):
                        for hcd in range(hq * 4, hq * 4 + 4):
                            nc.sync.dma_start(
                                out=wqkv_sb[:, hcd, :], in_=wqkv_d[:, hcd, :])
                        nc.sync.dma_start(
                            out=xts[0][:, hq * 4:(hq + 1) * 4, :],
                            in_=xt_d[ti0, :, hq * 4:(hq + 1) * 4, :])
                    nc.sync.dma_start(out=cs_sb, in_=cs_d[:, :, :])
                    for t in range(1, 4):
                        for hq in range(4):
                            nc.sync.dma_start(
                                out=xts[t][:, hq * 4:(hq + 1) * 4, :],
                                in_=xt_d[ti0 + t, :, hq * 4:(hq + 1) * 4, :])
                for t in range(4):
                    xt_tiles[ti0 + t] = xts[t]

            def proj_stream(bi):
                """QKV projection + RoPE + transpose for one 512-tok block.
                Yields after each PE instruction (interleave granularity)."""
                b, qb = blocks[bi]
                for t4 in range(4):
                    ti = b * TPB + qb * 4 + t4
                    xt = xt_tiles.pop(ti)
                    qk_ps = qkpool.tile([128, 512], f32, tag="qk")
                    for hc in range(HC):
                        nc.tensor.matmul(
                            qk_ps, xt[:, hc, :], wqkv_sb[:, hc, 0:512],
                            start=(hc == 0), stop=(hc == HC - 1))
                        yield
                    vh = v_ps[:, ti % 2, :]
                    for hc in range(HC):
                        nc.tensor.matmul(
                            vh, xt[:, hc, :], wqkv_sb[:, hc, 512:768],
                            start=(hc == 0), stop=(hc == HC - 1))
                        yield
                    # V -> SBUF (bf16), ones col already set
                    nc.vector.tensor_copy(
                        V_sb[:, ti, :, 0:64],
                        vh.rearrange("p (h d) -> p h d", d=64))
                    # qk -> SBUF bf16
                    qk = qpool.tile([128, 512], bf16, tag="qk")
                    nc.vector.tensor_copy(qk, qk_ps)
                    # partial RoPE on dims 0..15 of the 8 (q/k, head) blocks
                    rot = qk.rearrange("p (b d) -> p b d", d=64)[:, :, 0:ROT]
                    rot_lo = qk.rearrange("p (b d) -> p b d", d=64)[:, :, 0:8]
                    rot_hi = qk.rearrange("p (b d) -> p b d", d=64)[:, :, 8:16]
                    cos_bc = cs_sb[:, ti, None, 0:ROT].broadcast_to([128, 8, ROT])
                    sin_lo = cs_sb[:, ti, None, ROT:ROT + 8].broadcast_to([128, 8, 8])
                    sin_hi = cs_sb[:, ti, None, ROT + 8:ROT + 16].broadcast_to([128, 8, 8])
                    tmp = rpool.tile([128, 8, ROT], bf16, tag="t0")
                    t2l = rpool.tile([128, 8, 8], bf16, tag="t1")
                    t2h = rpool.tile([128, 8, 8], bf16, tag="t2")
                    nc.vector.tensor_mul(tmp, rot, cos_bc)
                    nc.vector.tensor_mul(t2l, rot_hi, sin_lo)
                    nc.vector.tensor_mul(t2h, rot_lo, sin_hi)
                    nc.vector.tensor_sub(rot_lo, tmp[:, :, 0:8], t2l)
                    nc.vector.tensor_add(rot_hi, tmp[:, :, 8:16], t2h)
                    # transpose the 4 dim-tiles into qkT
                    # all copies on DVE: a scalar.copy here would head-of-line
                    # block the exp stream on ScalarE's FIFO queue
                    tp = tpool.tile([128, 4, 128], bf16, tag="tp")
                    for dt in range(4):
                        nc.tensor.transpose(
                            tp[:, dt, :], qk[:, dt * 128:(dt + 1) * 128], ident)
                        nc.vector.tensor_copy(
                            qkT_sb[:, dt, ti * 128:(ti + 1) * 128],
                            tp[:, dt, :])
                        yield

            def oproj_stream(bi, use_act=False):
                """Out-projection for one completed 512-tok block."""
                b, qb = blocks[bi]
                for t4 in range(4):
                    ti = b * TPB + qb * 4 + t4
                    tp = tpool.tile([128, 4, 128], bf16, tag="tp")
                    nc.tensor.transpose(
                        tp[:, 0, :],
                        attnN_sb[:, 0, ti * 128:(ti + 1) * 128], ident)
                    nc.tensor.transpose(
                        tp[:, 1, :],
                        attnN_sb[:, 1, ti * 128:(ti + 1) * 128], ident)
                    aT = anpool.tile([128, 2, 128], bf16, tag="an")
                    nc.vector.tensor_copy(aT, tp[:, 0:2, :])
                    yield
                    for oc in range(4):
                        ops = opool.tile([128, 512], f32, tag="o")
                        nc.tensor.matmul(
                            ops, aT[:, 0, :],
                            wo_sb[:, 0, oc * 512:(oc + 1) * 512],
                            start=True, stop=False)
                        yield
                        nc.tensor.matmul(
                            ops, aT[:, 1, :],
                            wo_sb[:, 1, oc * 512:(oc + 1) * 512],
                            start=False, stop=True)
                        yield
                        ob = obpool.tile([128, 512], bf16, tag="ob")
                        if use_act and oc % 2 == 1:
                            nc.scalar.copy(ob, ops)
                        else:
                            nc.vector.tensor_copy(ob, ops)
                        nc.sync.dma_start(
                            out=out_d[ti * 128:(ti + 1) * 128,
                                      oc * 512:(oc + 1) * 512],
                            in_=ob)

            fills = []
            pending = [0]          # fill units outstanding
            pace = [0.0, 1.0]      # [accumulator, units-per-slot rate]

            def add_fill(gen, units):
                fills.append(gen)
                pending[0] += units

            def pull(n):
                """Advance fill generators round-robin, n units total."""
                for _ in range(n):
                    while fills:
                        try:
                            next(fills[0])
                            pending[0] -= 1
                            fills.append(fills.pop(0))
                            break
                        except StopIteration:
                            fills.pop(0)
                    if not fills:
                        return

            def paced_pull(weight):
                """Greedy: rationing fills was tried and LOST ~70us — the PE
                p-state ramp rewards continuous saturation, so front-loading
                fill work beats spreading it."""
                pull(weight)

            def drain():
                while fills:
                    pull(1)

            def force_finish(g):
                """Emit every remaining unit of one generator now."""
                n = 0
                for _ in g:
                    n += 1
                pending[0] -= n

            proj_gens = {}
            VJ = [10, 26, 42, 58]   # sum of valid (ki, j) pairs per qb

            # ---------------- prologue: project block 0 standalone
            issue_startup_dmas()
            issue_block_dmas(1, hp=True)
            wo_loaded = [False]
            proj_gens[0] = proj_stream(0)
            for _ in proj_gens[0]:
                pass

            nc.sync.dma_start(out=wo_sb, in_=wo_d[:, :, :])

            # ---------------- unified stream
            for bi, (b, qb) in enumerate(blocks):
                if bi + 1 < len(blocks):
                    if bi + 2 < len(blocks):
                        issue_block_dmas(bi + 2)
                    proj_gens[bi + 1] = proj_stream(bi + 1)
                    add_fill(proj_gens[bi + 1], 144)
                # attention(bi) depends on projection(bi): its instructions
                # must already be emitted (program order defines deps)
                if bi in proj_gens:
                    force_finish(proj_gens.pop(bi))
                # pace fills across this block's interleave slots
                slots = 2 * ((4 * qb + 4) * 2 + 2 * VJ[qb] + 8)
                pace[1] = pending[0] / slots if slots else 1.0
                pace[0] = 0.0
                for pr in range(2):          # head pairs (2pr, 2pr+1)
                    hA, hB = 2 * pr, 2 * pr + 1
                    accA = apool.tile([128, 4, 65], f32, tag="acc")
                    accB = apool.tile([128, 4, 65], f32, tag="acc")
                    with tc.high_priority(offset=150):
                        nc.tensor.matmul(
                            accA, ident, zeros_sb[:, 0:260],
                            start=True, stop=False)
                        nc.tensor.matmul(
                            accB, ident, zeros_sb[:, 0:260],
                            start=True, stop=False)
                    for ki in range(4 * qb + 4):
                        off = max(0, ki * 128 - qb * 512)
                        kcol = b * S + ki * 128
                        qcol = b * S + qb * 512
                        # one 2-bank tile + one exp per step: both score
                        # matmuls wait on the same exp, so the tile_position
                        # pair always runs back-to-back (473ns vs 2x318 split)
                        sAB = spool.tile([128, 2, 512], f32, tag="s")
                        with tc.high_priority(offset=150):
                            nc.tensor.matmul(
                                sAB[:, 0, off:512],
                                qkT_sb[0:64, 2 + pr, kcol:kcol + 128],
                                qkT_sb[0:64, pr, qcol + off:qcol + 512],
                                start=True, stop=True,
                                tile_position=(0, 0))
                            nc.tensor.matmul(
                                sAB[:, 1, off:512],
                                qkT_sb[64:128, 2 + pr, kcol:kcol + 128],
                                qkT_sb[64:128, pr, qcol + off:qcol + 512],
                                start=True, stop=True,
                                tile_position=(64, 0))
                        pAB = ppool.tile([128, 2, 512], bf16, tag="p")
                        nc.scalar.activation(
                            out=pAB[:, :, off:512], in_=sAB[:, :, off:512],
                            func=Exp)
                        if ki * 128 >= qb * 512:  # in-block diagonal
                            mask2 = mask_sb[:, None, :].broadcast_to(
                                [128, 2, 128])
                            nc.vector.tensor_mul(
                                pAB[:, :, off:off + 128],
                                pAB[:, :, off:off + 128], mask2)
                        paced_pull(2)
                        for j in range(4):
                            qg = qb * 4 + j
                            if qg < ki:
                                continue
                            last = (j == 3 and ki == 4 * qb + 3)
                            nc.tensor.matmul(
                                accA[:, j, 0:65],
                                pAB[:, 0, j * 128:(j + 1) * 128],
                                V_sb[:, b * TPB + ki, hA, 0:65],
                                start=False, stop=last)
                            paced_pull(1)
                            nc.tensor.matmul(
                                accB[:, j, 0:65],
                                pAB[:, 1, j * 128:(j + 1) * 128],
                                V_sb[:, b * TPB + ki, hB, 0:65],
                                start=False, stop=last)
                            paced_pull(1)
                    # normalize into attnN (q x dims layout)
                    last_block = (bi == len(blocks) - 1)
                    inline_op = None
                    if False and last_block and pr == 1:
                        inline_op = oproj_stream(bi, use_act=True)
                    for j in range(4):
                        recA = recpool.tile([128, 1], f32, tag="r")
                        recB = recpool.tile([128, 1], f32, tag="r")
                        nc.vector.reciprocal(recA, accA[:, j, 64:65])
                        nc.vector.reciprocal(recB, accB[:, j, 64:65])
                        col = b * S + (qb * 4 + j) * 128
                        nc.vector.tensor_scalar_mul(
                            attnN_sb[:, pr, col:col + 64],
                            accA[:, j, 0:64], recA)
                        nc.vector.tensor_scalar_mul(
                            attnN_sb[:, pr, col + 64:col + 128],
                            accB[:, j, 0:64], recB)
                        if inline_op is not None:
                            # last block: emit this q-tile's out-projection
                            # right after its attnN columns finalize
                            for _ in range(9):
                                try:
                                    next(inline_op)
                                except StopIteration:
                                    break
                        else:
                            paced_pull(2)
                    if inline_op is not None:
                        for _ in inline_op:
                            pass
                # out-projection of this block interleaves into the next
                # block's attention; the last block was emitted inline above
                if not last_block:
                    add_fill(oproj_stream(bi), 36)
                else:
                    add_fill(oproj_stream(bi, use_act=True), 36)
                    drain()

            for p in reversed(ps_pools):
                p.__exit__(None, None, None)
            for p in reversed(sb_pools):
                p.__exit__(None, None, None)
    nc.finalize()
    return nc


# --------------------------------------------------------------------------
# Host-side prep
# --------------------------------------------------------------------------

def _host_prep(hidden_states, qkv_w, o_w, position_ids, S=S_FULL):
    """Returns (shared dict, per-core list of dicts) of numpy arrays."""
    T = B * S
    TT = T // 128
    HC = H // 128
    x = np.asarray(hidden_states, dtype=np.float32).reshape(T, H)
    # pre-tiled xT: [TT, 128(p=h%128), HC, 128(tok)] contiguous per DMA tile
    xtl = np.ascontiguousarray(
        x.reshape(TT, 128, HC, 128).transpose(0, 3, 2, 1)).astype(nbf16)

    pos = np.asarray(position_ids).reshape(T).astype(np.float64)
    inv = THETA ** (-np.arange(0, ROT, 2, dtype=np.float64) / ROT)  # [8]
    f = pos[:, None] * inv[None, :]                                 # [T, 8]
    emb = np.concatenate([f, f], axis=1)                            # [T, 16]
    # packed per-partition-linear layout [128, TT, 32]: cos | sin
    cs = np.empty((128, TT, 2 * ROT), np.float32)
    cs[:, :, 0:ROT] = np.cos(emb).reshape(TT, 128, ROT).transpose(1, 0, 2)
    cs[:, :, ROT:2 * ROT] = np.sin(emb).reshape(TT, 128, ROT).transpose(1, 0, 2)
    csd = np.ascontiguousarray(cs).astype(nbf16)

    # mask[p, j]: 1 when q offset j >= k offset p else 0 (applied post-exp)
    p_idx = np.arange(128)[:, None]
    j_idx = np.arange(128)[None, :]
    maskd = np.ascontiguousarray(
        np.where(j_idx >= p_idx, 1.0, 0.0)).astype(nbf16)

    shared = {"xtl": xtl, "csd": csd, "maskd": maskd}

    qkv = np.asarray(qkv_w, dtype=np.float32)
    ow = np.asarray(o_w, dtype=np.float32)
    scale = 1.0 / np.sqrt(HD)
    per_core = []
    for c in range(NCORES):
        cols = np.empty((768, H), np.float32)
        for t in range(4):                    # qk dim-tiles
            qk_sel = 0 if t < 2 else 1        # 0 = q, 1 = k
            for u in range(2):
                hl = 2 * (t % 2) + u
                hg = HPC * c + hl
                w = qkv[qk_sel * H + hg * HD: qk_sel * H + (hg + 1) * HD]
                if qk_sel == 0:
                    w = w * scale
                cols[t * 128 + u * 64: t * 128 + u * 64 + 64] = w
        for hl in range(HPC):                 # v dims
            hg = HPC * c + hl
            cols[512 + hl * 64: 512 + (hl + 1) * 64] = \
                qkv[2 * H + hg * HD: 2 * H + (hg + 1) * HD]
        wqkvT = cols.T                        # [H, 768]
        wqkvl = np.ascontiguousarray(
            wqkvT.reshape(HC, 128, 768).transpose(1, 0, 2)).astype(nbf16)
        woT = ow[:, LDIM * c: LDIM * (c + 1)].T   # [256, H]
        wol = np.ascontiguousarray(
            woT.reshape(2, 128, H).transpose(1, 0, 2)).astype(nbf16)
        per_core.append({"wqkvl": wqkvl, "wol": wol})
    return shared, per_core


_NC_CACHE = {}


def _get_nc(S=S_FULL):
    if S not in _NC_CACHE:
        _NC_CACHE[S] = build_nc(S)
    return _NC_CACHE[S]


def _run(hidden_states, qkv_w, o_w, position_ids, S=S_FULL, trace=False,
         trace_kwargs=None):
    shared, per_core = _host_prep(hidden_states, qkv_w, o_w, position_ids, S)
    in_maps = [{**shared, **per_core[c]} for c in range(NCORES)]
    nc = _get_nc(S)
    br = run_bass_kernel_spmd(
        nc, in_maps, list(range(NCORES)), trace=trace,
        **(trace_kwargs or {}))
    T = B * S
    out = np.zeros((T, H), np.float32)
    for r in br.results:
        out += r["out"].astype(np.float32)
    return out.reshape(B, S, H), br


def kernel(hidden_states, qkv_w, o_w, position_ids):
    out, _ = _run(hidden_states, qkv_w, o_w, position_ids)
    return out
